# revision 1
# baseline (speedup 1.0000x reference)
"""GATConv (single-head, PyG defaults) on 8 Trainium2 NeuronCores.

Strategy (dst-sharded, host does indexing only, device does all FLOPs):
  - Destinations are sharded 8 ways (6250 nodes/core), windows of 32
    consecutive dst nodes; each window's incoming edges (incl. self-loops)
    are padded to 128-edge tiles.
  - Per tile the host ships a PE-ready stacked lhsT [128, 128] fp16:
    rows 0..95 = x[src_e]^T (the halo-gathered source features, pure
    host-side indexing of the input), rows 96..127 = one-hot of the
    window-local destination (indT).  One matmul against
    R = [[W | v_src], [0 | a_dst-col]] produces h[src_e] AND the edge score
    e = a_src[src] + a_dst[dst] in one pass (v_src = W @ att_src is computed
    on device; the a_dst block of R is refreshed per window from a tiny
    x[dst]^T matmul).
  - w = exp(leaky_relu(e) - 4) on ScalarE (the -4 keeps fp16 in range and
    cancels in the softmax); G_w = h * w via per-partition-scalar copies
    (split across VectorE/ScalarE); segment softmax + aggregation happen in
    one accumulating PE matmul per tile: psum[s,:] += ind^T @ [G_w | w],
    whose col 96 accumulates the softmax denominator.
  - Epilogue per 4 windows: out = tanh(num/den + bias) -> HBM.
No device-side gather/scatter instructions are needed; all traffic is
sequential DMA.
"""

import numpy as np

import concourse.bass as bass
import concourse.mybir as mybir
import concourse.tile as tile
from concourse.vector_clock import ScopedClock
from concourse.bass_utils import run_bass_kernel_spmd

# ----------------------------------------------------------------------------
# walrus workaround: this toolchain rejects >1 sync-wait per instruction.
# Split multi-wait instructions into same-engine NOPs carrying one wait each.
# ----------------------------------------------------------------------------
_PATCHED = False


def _install_tile_patches():
    global _PATCHED
    if _PATCHED:
        return
    _PATCHED = True
    orig_lower = tile.TileContext._lower_ordered_insts
    ctr = [0]

    def _spill(insts):
        out = []
        for inst in insts:
            si = getattr(inst, "sync_info", None)
            n_w = len(si.on_wait) if si is not None else 0
            if n_w > 1 and not bass.is_branch_inst(inst):
                waits = list(si.on_wait)
                for w in waits[:-1]:
                    ctr[0] += 1
                    nop = mybir.InstNoOp(name=f"I-waitspill-{ctr[0]}", ins=[], outs=[])
                    nop.engine = inst.engine
                    nop.bass_nofuse = True
                    nop.sync_info = mybir.SyncInfo(on_wait=[w], on_update=[])
                    out.append(nop)
                inst.sync_info = mybir.SyncInfo(
                    on_wait=[waits[-1]], on_update=list(si.on_update)
                )
            out.append(inst)
        return out

    def _patched_lower(self, ordered):
        for bb in list(ordered.keys()):
            ordered[bb] = _spill(ordered[bb])
        return orig_lower(self, ordered)

    def _patched_drain(self, tick_clock, wait_clock):
        nc = self.nc
        probe = nc.sync.nop(nofuse=True)
        wait_clock.add_sem_waits(
            probe.ins, ScopedClock({None: tick_clock.global_clock})
        )
        si = probe.ins.sync_info
        waits = list(si.on_wait) if si is not None else []
        probe.ins.sync_info = mybir.SyncInfo(
            on_wait=waits[:1], on_update=list(si.on_update) if si else []
        )
        for w in waits[1:]:
            n2 = nc.sync.nop(nofuse=True)
            n2.ins.sync_info = mybir.SyncInfo(on_wait=[w], on_update=[])
        nc.sync.drain()
        nc.all_engine_barrier()
        popped = nc._tile_sem_poison_stack.pop()
        assert popped is self._sem_poison
        nc.clear_and_free_semaphores(list(self.sems.allocated().values()))
        nc.all_engine_barrier()

    tile.TileContext._lower_ordered_insts = _patched_lower
    tile.TileContext._drain_and_barrier = _patched_drain


# ----------------------------------------------------------------------------
# problem constants (hardcoded per the harness contract)
# ----------------------------------------------------------------------------
N_NODES = 50000
N_CORES = 8
D = 96
WIN = 32          # dst nodes per window (indT rows: 96 + 32 = 128 partitions)
P = 128
GRP = 8           # tiles per staging group
NEG_SLOPE = 0.2
EXP_BIAS = -4.0   # global shift inside exp(); cancels in the softmax
F16 = mybir.dt.float16
F32 = mybir.dt.float32


def _preprocess(x, edge_index):
    """Sort/group/pad edges, gather x rows host-side, build shipped tensors."""
    shard = N_NODES // N_CORES
    n_win = (shard + WIN - 1) // WIN          # 196
    assert n_win % 4 == 0
    src = np.concatenate(
        [np.asarray(edge_index[0], dtype=np.int64), np.arange(N_NODES)]
    )
    dst = np.concatenate(
        [np.asarray(edge_index[1], dtype=np.int64), np.arange(N_NODES)]
    )
    order = np.argsort(dst, kind="stable")
    src, dst = src[order], dst[order]
    core_of = dst // shard
    x16 = np.asarray(x, dtype=np.float16)

    per_core_edges = []
    T_w = np.zeros(n_win, dtype=np.int64)
    for c in range(N_CORES):
        m = core_of == c
        s, d = src[m], dst[m] - c * shard
        per_core_edges.append((s, d))
        cnt = np.bincount(d // WIN, minlength=n_win)
        T_w = np.maximum(T_w, (cnt + P - 1) // P)
    T_w = np.maximum(T_w, 1)
    # pad total tile count to a multiple of GRP by extending the last window
    tot = int(T_w.sum())
    T_w[-1] += (-tot) % GRP
    tot = int(T_w.sum())
    n_grp = tot // GRP

    # per-tile window id (same for all cores)
    tile_win = np.repeat(np.arange(n_win), T_w)

    per_core = []
    for c in range(N_CORES):
        s, d = per_core_edges[c]
        wid = d // WIN
        stacked = np.zeros((tot, P, P), np.float16)
        ind = np.zeros((tot, P, WIN), np.float16)
        t0 = 0
        for w in range(n_win):
            m = wid == w
            sw = s[m]
            dw = d[m] - w * WIN
            ne = len(sw)
            nt = int(T_w[w])
            xg = np.zeros((nt * P, D), np.float16)
            xg[:ne] = x16[sw]
            oh = np.zeros((nt * P, WIN), np.float16)
            oh[np.arange(ne), dw] = 1.0
            st = stacked[t0:t0 + nt]
            st[:, 0:D, :] = xg.reshape(nt, P, D).transpose(0, 2, 1)
            st[:, D:D + WIN, :] = oh.reshape(nt, P, WIN).transpose(0, 2, 1)
            ind[t0:t0 + nt] = oh.reshape(nt, P, WIN)
            t0 += nt
        # group-major DMA layout: [n_grp, 128, GRP*128] / [n_grp, 128, GRP*32]
        stacked_g = (
            stacked.reshape(n_grp, GRP, P, P).transpose(0, 2, 1, 3)
            .reshape(n_grp, P, GRP * P).copy()
        )
        ind_g = (
            ind.reshape(n_grp, GRP, P, WIN).transpose(0, 2, 1, 3)
            .reshape(n_grp, P, GRP * WIN).copy()
        )
        # xd4: x[dst nodes]^T per 4-window block -> [n_win//4, 96, 128]
        ids = np.arange(n_win * WIN) + c * shard
        valid = ids < (c + 1) * shard
        ids = np.minimum(ids, N_NODES - 1)
        xdT = x16[ids].T.copy()              # [96, n_win*WIN]
        xdT[:, ~valid] = 0
        xd4 = (
            xdT.reshape(D, n_win // 4, 4 * WIN).transpose(1, 0, 2).copy()
        )
        per_core.append(dict(stacked=stacked_g, ind=ind_g, xd4=xd4))
    return per_core, T_w, tile_win, shard, n_win, n_grp


def _build(T_w, tile_win, n_win, n_grp):
    _install_tile_patches()
    n_wg = n_win // 4
    tot = n_grp * GRP
    nc = bass.Bass("TRN2", target_bir_lowering=False, debug=False, num_devices=1)
    stacked_in = nc.declare_dram_parameter(
        "stacked", [n_grp, P, GRP * P], F16, isOutput=False)
    ind_in = nc.declare_dram_parameter(
        "ind", [n_grp, P, GRP * WIN], F16, isOutput=False)
    xd4_in = nc.declare_dram_parameter("xd4", [n_wg, D, 4 * WIN], F16, isOutput=False)
    w_in = nc.declare_dram_parameter("wmat", [D, D], F32, isOutput=False)
    asrc_in = nc.declare_dram_parameter("att_src", [D, D], F32, isOutput=False)
    adst_in = nc.declare_dram_parameter("att_dst", [D, D], F32, isOutput=False)
    bias_in = nc.declare_dram_parameter("bias", [P, D], F32, isOutput=False)
    out_t = nc.declare_dram_parameter("out", [n_wg * P, D], F32, isOutput=True)

    # per-tile metadata
    win_of = tile_win                      # window id per tile
    first_tile = np.zeros(n_win, np.int64)
    last_tile = np.zeros(n_win, np.int64)
    for w in range(n_win):
        idxs = np.where(win_of == w)[0]
        first_tile[w], last_tile[w] = idxs[0], idxs[-1]

    with tile.TileContext(nc) as tc:
        with (
            tc.tile_pool(name="const", bufs=1) as cpool,
            tc.tile_pool(name="st8", bufs=3) as st_pool,
            tc.tile_pool(name="ind8", bufs=3) as ind_pool,
            tc.tile_pool(name="gw8", bufs=2) as gw_pool,
            tc.tile_pool(name="small", bufs=3) as sm_pool,
            tc.tile_pool(name="rwin", bufs=3) as r_pool,
            tc.tile_pool(name="xd", bufs=2) as xd_pool,
            tc.tile_pool(name="ep", bufs=2) as ep_pool,
            tc.tile_pool(name="pg8", bufs=2, space="PSUM") as pg_pool,
            tc.tile_pool(name="pwin", bufs=2, space="PSUM") as pw_pool,
            tc.tile_pool(name="padst", bufs=2, space="PSUM") as pa_pool,
        ):
            # ---- pre-phase: v_src/v_dst, Wext, bias ----
            w_sb = cpool.tile([D, D], F32)
            nc.sync.dma_start(out=w_sb[:], in_=w_in[:, :])
            asrc_rep = cpool.tile([D, D], F32)
            nc.sync.dma_start(out=asrc_rep[:], in_=asrc_in[:, :])
            adst_rep = cpool.tile([D, D], F32)
            nc.sync.dma_start(out=adst_rep[:], in_=adst_in[:, :])
            bias_rep = cpool.tile([P, D], F32)
            nc.sync.dma_start(out=bias_rep[:], in_=bias_in[:, :])

            tmp = cpool.tile([D, D], F32)
            vsrc = cpool.tile([D, 1], F32)
            vdst16 = cpool.tile([D, 1], F16)
            nc.vector.tensor_tensor(
                out=tmp[:], in0=w_sb[:], in1=asrc_rep[:],
                op=mybir.AluOpType.mult)
            nc.vector.tensor_reduce(
                out=vsrc[:], in_=tmp[:], axis=mybir.AxisListType.X,
                op=mybir.AluOpType.add)
            nc.vector.tensor_tensor(
                out=tmp[:], in0=w_sb[:], in1=adst_rep[:],
                op=mybir.AluOpType.mult)
            vdst = cpool.tile([D, 1], F32)
            nc.vector.tensor_reduce(
                out=vdst[:], in_=tmp[:], axis=mybir.AxisListType.X,
                op=mybir.AluOpType.add)
            nc.vector.tensor_copy(out=vdst16[:], in_=vdst[:])

            wext = cpool.tile([P, D + 1], F16)      # [128, 97]
            nc.vector.memset(wext[:], 0.0)
            nc.vector.tensor_copy(out=wext[0:D, 0:D], in_=w_sb[:])
            nc.vector.tensor_copy(out=wext[0:D, D:D + 1], in_=vsrc[:])

            neg4 = cpool.tile([P, 1], F32)
            nc.vector.memset(neg4[:], EXP_BIAS)

            # ---- main ----
            r_tiles = {}          # window -> R tile
            pw_tiles = {}         # wg -> psum win tile
            adst_tiles = {}       # wg -> adst sbuf tile
            alt = 0

            for g in range(n_grp):
                st8 = st_pool.tile([P, GRP, P], F16, tag="st8")
                nc.sync.dma_start(
                    out=st8[:].rearrange("p a b -> p (a b)"), in_=stacked_in[g, :, :])
                ind8 = ind_pool.tile([P, GRP, WIN], F16, tag="ind8")
                nc.sync.dma_start(
                    out=ind8[:].rearrange("p a b -> p (a b)"), in_=ind_in[g, :, :])
                g8 = pg_pool.tile([P, GRP, P], F32, tag="pg8")
                gw8 = gw_pool.tile([P, GRP, D + 1], F16, tag="gw8")
                t8 = sm_pool.tile([P, GRP], F32, tag="t8")
                u8 = sm_pool.tile([P, GRP], F32, tag="u8")
                w8 = sm_pool.tile([P, GRP], F32, tag="w8")

                # pass 1: combined feature+score matmuls
                for j in range(GRP):
                    t = g * GRP + j
                    w = int(win_of[t])
                    wg = w // 4
                    if wg not in pw_tiles:
                        # new 4-window block: a_dst matmul
                        xd_t = xd_pool.tile([D, 4 * WIN], F16, tag="xd")
                        nc.sync.dma_start(out=xd_t[:], in_=xd4_in[wg, :, :])
                        pa = pa_pool.tile([P, 1], F32, tag="pa")
                        nc.tensor.matmul(
                            out=pa[:], lhsT=xd_t[:], rhs=vdst16[:],
                            start=True, stop=True)
                        adst4 = sm_pool.tile([P, 1], F16, tag="adst")
                        nc.scalar.activation(
                            out=adst4[:], in_=pa[:],
                            func=mybir.ActivationFunctionType.Copy)
                        adst_tiles[wg] = adst4
                        pw_tiles[wg] = pw_pool.tile([P, P], F32, name=f"pw{wg}", tag="pw")
                    if w not in r_tiles:
                        R = r_pool.tile([P, D + 1], F16, tag="rw")
                        nc.gpsimd.tensor_copy(out=R[:], in_=wext[:])
                        j4 = w % 4
                        nc.vector.tensor_copy(
                            out=R[D:D + WIN, D:D + 1],
                            in_=adst_tiles[w // 4][WIN * j4:WIN * (j4 + 1), :])
                        r_tiles[w] = R
                    nc.tensor.matmul(
                        out=g8[:, j, 0:D + 1], lhsT=st8[:, j, :],
                        rhs=r_tiles[w][:], start=True, stop=True)

                # group scalar phase: w = exp(lrelu(e) - 4)
                # (ACT's Lrelu table ignores alpha; do max(x, 0.2x) on DVE)
                nc.vector.tensor_scalar_mul(
                    out=t8[:], in0=g8[:, :, D], scalar1=NEG_SLOPE)
                nc.vector.tensor_tensor(
                    out=u8[:], in0=t8[:], in1=g8[:, :, D],
                    op=mybir.AluOpType.max)
                nc.scalar.activation(
                    out=w8[:], in_=u8[:],
                    func=mybir.ActivationFunctionType.Exp, bias=neg4[:])
                nc.vector.tensor_copy(out=gw8[:, :, D], in_=w8[:])

                # pass 2: weight rows + aggregate
                for j in range(GRP):
                    t = g * GRP + j
                    w = int(win_of[t])
                    wg = w // 4
                    if alt == 0:
                        nc.vector.tensor_scalar(
                            out=gw8[:, j, 0:D], in0=g8[:, j, 0:D],
                            scalar1=w8[:, j:j + 1], scalar2=None,
                            op0=mybir.AluOpType.mult)
                    else:
                        nc.scalar.activation(
                            out=gw8[:, j, 0:D], in_=g8[:, j, 0:D],
                            func=mybir.ActivationFunctionType.Copy,
                            scale=w8[:, j:j + 1])
                    alt ^= 1
                    pw = pw_tiles[wg]
                    j4 = w % 4
                    nc.tensor.matmul(
                        out=pw[WIN * j4:WIN * (j4 + 1), 0:D + 1],
                        lhsT=ind8[:, j, :], rhs=gw8[:, j, 0:D + 1],
                        start=(t == first_tile[w]), stop=(t == last_tile[w]),
                        tile_position=(0, WIN * j4))
                    # epilogue when the last window of a 4-block completes
                    if t == last_tile[w] and w % 4 == 3:
                        den = ep_pool.tile([P, 1], F32, tag="den")
                        rcp = ep_pool.tile([P, 1], F32, tag="rcp")
                        res = ep_pool.tile([P, D], F32, tag="res")
                        outb = ep_pool.tile([P, D], F32, tag="outb")
                        nc.vector.tensor_scalar_add(
                            out=den[:], in0=pw[:, D:D + 1], scalar1=1e-9)
                        nc.vector.reciprocal(out=rcp[:], in_=den[:])
                        nc.vector.scalar_tensor_tensor(
                            out=res[:], in0=pw[:, 0:D], scalar=rcp[:],
                            in1=bias_rep[:],
                            op0=mybir.AluOpType.mult, op1=mybir.AluOpType.add)
                        nc.scalar.activation(
                            out=outb[:], in_=res[:],
                            func=mybir.ActivationFunctionType.Tanh)
                        nc.sync.dma_start(
                            out=out_t[wg * P:(wg + 1) * P, :], in_=outb[:])
                        del pw_tiles[wg]
                        del adst_tiles[wg]
                    if t == last_tile[w]:
                        del r_tiles[w]
    return nc


_CACHE = {}


def kernel(x, W, att_src, att_dst, bias, edge_index):
    x = np.asarray(x)
    W = np.asarray(W, dtype=np.float32)
    att_src = np.asarray(att_src, dtype=np.float32)
    att_dst = np.asarray(att_dst, dtype=np.float32)
    bias = np.asarray(bias, dtype=np.float32)
    per_core, T_w, tile_win, shard, n_win, n_grp = _preprocess(x, edge_index)

    key = (n_grp, tuple(T_w.tolist()))
    if key not in _CACHE:
        _CACHE[key] = _build(T_w, tile_win, n_win, n_grp)
    nc = _CACHE[key]

    in_maps = []
    for c in range(N_CORES):
        pc = per_core[c]
        in_maps.append({
            "stacked": pc["stacked"],
            "ind": pc["ind"],
            "xd4": pc["xd4"],
            "wmat": W,
            "att_src": np.tile(att_src.reshape(1, D), (D, 1)),
            "att_dst": np.tile(att_dst.reshape(1, D), (D, 1)),
            "bias": np.tile(bias.reshape(1, D), (P, 1)),
        })
    res = run_bass_kernel_spmd(nc, in_maps, list(range(N_CORES)))
    outs = [res.results[c]["out"][:shard] for c in range(N_CORES)]
    return np.concatenate(outs, axis=0).astype(np.float32)

